# revision 1
# baseline (speedup 1.0000x reference)
"""Causal self-attention (B=2, T=2048, C=768, H=12, DH=64) on 8 TRN2 cores.

Sharding: core = (b, g), b in {0,1} batch, g in {0..3} head-group of 3
heads.  Data parallel on B, tensor parallel on H: Wqkv column-sharded,
Wout row-sharded; the 4 partial outputs per batch are summed on the host
(the all-reduce of the row-parallel projection).

Device kernel (per core), bf16 compute / f32 PSUM:
  - qkT [384, T] = Wqk_shard.T @ x.T with W col order [q0 q2|k0 k2|q1 k1]:
    every head's scores matmul finds its q/k slice pair sharing a base
    partition (a matmul operand requirement) with zero extra m-tiles;
    V is no longer part of this matmul.
  - qk_s[3] = [k1|q1] (partition-swapped copy of m2 via SBUF-SBUF DMA)
    gives head 1 partition-aligned q/k slices on both halves; its
    q-range alternates between them per chunk.
  - V is computed directly in [t, dh] orientation (xT-chunk stationary,
    Wv moving), which is exactly the AV layout -- no PE transposes.
  - scores are computed transposed S^T [k, q], causal-chunked; exp on
    ScalarE (its only job) reads PSUM and writes P^T bf16 tiles.
  - V' = V with ones column appended: col 64 of the AV accumulator is
    the softmax denominator; normalization = reciprocal + per-partition
    scale on DVE.
  - attn [q,192] is PE-transposed and projected through the Wout
    row-shard; partial [T, C] f32 is DMA'd out per q-tile.
"""

import os
import sys

sys.path.insert(0, "/root/.axon_site")
sys.path.insert(0, "/root/.axon_site/_ro/trn_rl_repo")
sys.path.insert(0, "/root/.axon_site/_ro/pypackages")

import numpy as np
import ml_dtypes

import concourse.bass as bass
import concourse.mybir as mybir
import concourse.tile as tile
import concourse.bacc as bacc
from concourse.bass_utils import run_bass_kernel_spmd

B, T, C, H, DH = 2, 2048, 768, 12, 64
G = 4                 # head groups (tensor parallel)
HPG = H // G          # 3 heads per group
CPG = HPG * DH        # 192 output cols per group
NCORES = B * G        # 8

F32 = mybir.dt.float32
BF16 = mybir.dt.bfloat16

NT = T // 128         # 16 t-tiles
NCS = C // 128        # 6 c-strips
SCALE = DH ** -0.5

_COMPILED = {}


def _patch_walrus_ldw_opt():
    """Re-enable walrus's LDWEIGHTS elision: consecutive matmuls with the
    same stationary operand then skip the redundant weight reloads."""
    from concourse import bass_utils
    if getattr(bass_utils, "_ldw_opt_patched", False):
        return
    orig = bass_utils.run_command

    def patched(cmd, **kw):
        cmd = ["--enable-ldw-opt=true" if c == "--enable-ldw-opt=false" else c
               for c in cmd]
        return orig(cmd, **kw)

    bass_utils.run_command = patched
    bass_utils._ldw_opt_patched = True


def _build_nc():
    from contextlib import ExitStack

    if os.environ.get("LDW_OPT", "0") == "1":
        _patch_walrus_ldw_opt()
    nc = bacc.Bacc("TRN2", debug=False, num_devices=NCORES)

    xt_d = nc.dram_tensor("xt", [C, T], BF16, kind="ExternalInput").ap()
    wqk_d = nc.dram_tensor("wqk", [C, 384], BF16, kind="ExternalInput").ap()
    wv_d = nc.dram_tensor("wv", [C, CPG], BF16, kind="ExternalInput").ap()
    wo_d = nc.dram_tensor("wo", [CPG, C], BF16, kind="ExternalInput").ap()
    id_d = nc.dram_tensor("ident", [128, 128], BF16, kind="ExternalInput").ap()
    mk_d = nc.dram_tensor("mask", [128, 128], BF16, kind="ExternalInput").ap()
    out_d = nc.dram_tensor("out", [T, C], F32, kind="ExternalOutput").ap()

    with tile.TileContext(nc) as tc, ExitStack() as ctx:
        _kernel(ctx, tc, nc, xt_d, wqk_d, wv_d, wo_d, id_d, mk_d, out_d)

    nc.compile()
    return nc


def _kernel(ctx, tc, nc, xt_d, wqk_d, wv_d, wo_d, id_d, mk_d, out_d):
    Exp = mybir.ActivationFunctionType.Exp

    # ---- persistent SBUF tensors -------------------------------------
    persist = ctx.enter_context(tc.tile_pool(name="persist", bufs=1))

    def single(shape, dtype, name):
        return persist.tile(shape, dtype, tag=name, name=name)

    xt_s = [single([128, T], BF16, f"xt{i}") for i in range(NCS)]
    wqk_s = [single([128, 384], BF16, f"wqk{i}") for i in range(NCS)]
    wv_s = [single([128, CPG], BF16, f"wv{i}") for i in range(NCS)]
    wo_hi = single([128, C], BF16, "wohi")
    wo_lo = single([CPG - 128, C], BF16, "wolo")
    ident = single([128, 128], BF16, "ident")
    maskt = single([128, 128], BF16, "maskt")
    # qkT m-tiles: qk_s[0]=[q0|q2] qk_s[1]=[k0|k2] qk_s[2]=[q1|k1]
    # qk_s[3]=[k1|q1] (partition-swapped copy of qk_s[2])
    qk_s = [single([128, T], BF16, f"qk{i}") for i in range(4)]
    # V' per head: [128 k-part, 16 k-tiles * 65] (col 64 of each 65 = 1.0)
    vp_s = [single([128, NT * 65], BF16, f"vp{h}") for h in range(HPG)]
    # attention output, q-tile j at cols [CPG*j : CPG*(j+1))
    ao_all = single([128, NT * CPG], BF16, "aoall")

    # ---- input DMAs: interleave strips so cs=0 inputs land first -----
    # first qkT matmul needs wqk strip 0 + xt strip 0 chunk 0 only.
    qdma = [nc.sync, nc.gpsimd]

    for cs in range(NCS):
        qdma[cs % 2].dma_start(wqk_s[cs][:, :], wqk_d[cs * 128:(cs + 1) * 128, :])
        qdma[(cs + 1) % 2].dma_start(
            xt_s[cs][:, 0:512], xt_d[cs * 128:(cs + 1) * 128, 0:512])
    for n4 in range(1, 4):
        for cs in range(NCS):
            qdma[(cs + n4) % 2].dma_start(
                xt_s[cs][:, n4 * 512:(n4 + 1) * 512],
                xt_d[cs * 128:(cs + 1) * 128, n4 * 512:(n4 + 1) * 512])
    nc.sync.dma_start(maskt[:, :], mk_d[:, :])
    for cs in range(NCS):
        qdma[cs % 2].dma_start(wv_s[cs][:, :], wv_d[cs * 128:(cs + 1) * 128, :])
    nc.sync.dma_start(ident[:, :], id_d[:, :])
    nc.sync.dma_start(wo_hi[:, :], wo_d[0:128, :])
    nc.sync.dma_start(wo_lo[:, :], wo_d[128:CPG, :])
    for h in range(HPG):
        nc.gpsimd.memset(vp_s[h][:, :], 1.0)

    # ---- PSUM pools (8 banks of 2KB/partition total) -----------------
    # scp: score chunks, [128,1024] f32 ring of 2           -> 4 banks
    # big: qkT chunks / direct-V / proj halves, ring of 2   -> 2 banks
    # sml: AV accumulators + transpose pair, ring of 2      -> 2 banks
    scp = ctx.enter_context(tc.tile_pool(name="scp", bufs=2, space="PSUM"))
    big = ctx.enter_context(tc.tile_pool(name="pbig", bufs=2, space="PSUM"))
    sml = ctx.enter_context(tc.tile_pool(name="psml", bufs=2, space="PSUM"))
    atp = ctx.enter_context(tc.tile_pool(name="atp", bufs=3))
    otp = ctx.enter_context(tc.tile_pool(name="otp", bufs=3))
    rcp = ctx.enter_context(tc.tile_pool(name="rcp", bufs=4))

    nn = [0]

    def psum_sc(p, f):
        t = scp.tile([p, f], F32, tag="sc", name=f"sc{nn[0]}",
                     padded_shape=[128, 1024])
        nn[0] += 1
        return t

    def psum_big(p, f):
        t = big.tile([p, f], F32, tag="big", name=f"bg{nn[0]}",
                     padded_shape=[128, 512])
        nn[0] += 1
        return t

    def psum_sm(p, f, dtype):
        t = sml.tile([p, f], dtype, tag="sm", name=f"sm{nn[0]}")
        nn[0] += 1
        return t

    # ---- qkT: m-tile chunk = 6-strip accumulation + PSUM->SBUF copy --
    def emit_qk_chunk(m, n4):
        ps = psum_big(128, 512)
        for cs in range(NCS):
            nc.tensor.matmul(
                ps[:, :],
                wqk_s[cs][:, m * 128:(m + 1) * 128],
                xt_s[cs][:, n4 * 512:(n4 + 1) * 512],
                start=(cs == 0), stop=(cs == NCS - 1),
            )
        nc.vector.tensor_copy(qk_s[m][:, n4 * 512:(n4 + 1) * 512], ps[:, :])

    # ---- direct V: V[t, dh] = xT-chunk.T @ Wv ------------------------
    def emit_v(t):
        pv = psum_big(128, CPG)
        for cs in range(NCS):
            nc.tensor.matmul(
                pv[:, :],
                xt_s[cs][:, t * 128:(t + 1) * 128],
                wv_s[cs][:, :],
                start=(cs == 0), stop=(cs == NCS - 1),
            )
        for h in range(HPG):
            nc.vector.tensor_copy(vp_s[h][:, 65 * t:65 * t + 64],
                                  pv[:, h * 64:(h + 1) * 64])

    # ---- scores lanes ------------------------------------------------
    # lane 0 = array rows 0:64, lane 1 = rows 64:128 (via base partition)
    # (qk tile idx, partition offset) for each head's qT / kT
    q_loc = {0: (0, 0), 2: (0, 64), "1lo": (2, 0), "1hi": (3, 64)}
    k_loc = {0: (1, 0), 2: (1, 64), "1lo": (3, 0), "1hi": (2, 64)}

    pt_all = [[None] * NT for _ in range(HPG)]

    def emit_scores_chunk(h, i, ci):
        """One [<=1024]-col chunk of head h's S^T tile i: matmul + exp.
        h in {0, 2} use their own lane; h == 1 alternates lanes per
        chunk (its q-range is split across both PE array halves)."""
        qlen = T - 128 * i
        pti = pt_all[h][i]
        q0 = 128 * i
        c0 = ci * 1024
        L = min(1024, qlen - c0)
        if h == 1:
            key = "1lo" if ci % 2 == 0 else "1hi"
            lane = ci % 2
        else:
            key = h
            lane = 0 if h == 0 else 1
        qt, qp = q_loc[key]
        kt, kp = k_loc[key]
        sc = psum_sc(128, L)
        for s0 in range(0, L, 512):
            sl = min(512, L - s0)
            nc.tensor.matmul(
                sc[:, s0:s0 + sl],
                qk_s[kt][kp:kp + 64, i * 128:(i + 1) * 128],
                qk_s[qt][qp:qp + 64, q0 + c0 + s0:q0 + c0 + s0 + sl],
                start=True, stop=True,
            )
        nc.scalar.activation(pti[:, c0:c0 + L], sc[:, :], Exp, scale=SCALE)
        if ci == 0:
            # zero the upper-triangular (k > q) part of the diag block
            nc.vector.tensor_mul(pti[:, 0:128], pti[:, 0:128], maskt[:, :])

    def emit_scores(h, i, chunks=None):
        qlen = T - 128 * i
        if pt_all[h][i] is None:
            pt_all[h][i] = single([128, qlen], BF16, f"pth{h}i{i}")
        nch = (qlen + 1023) // 1024
        for ci in (range(nch) if chunks is None else chunks):
            if ci < nch:
                emit_scores_chunk(h, i, ci)

    def emit_av(j, heads):
        """AV chains for `heads` into one accumulator (65 cols per head,
        col 64 of each = softmax denominator), one reciprocal for all
        denominators, one scaled copy per head."""
        po = psum_sm(128, len(heads) * 65, F32)
        for hi, h in enumerate(heads):
            for i in range(j + 1):
                nc.tensor.matmul(
                    po[:, 65 * hi:65 * hi + 65],
                    pt_all[h][i][:, (j - i) * 128:(j - i + 1) * 128],
                    vp_s[h][:, 65 * i:65 * i + 65],
                    start=(i == 0), stop=(i == j),
                )
        rec = rcp.tile([128, len(heads)], F32, tag="rc", name=f"rc{j}")
        nc.vector.reciprocal(rec[:, :], po[:, 64:65 * len(heads):65])
        for hi, h in enumerate(heads):
            nc.vector.tensor_scalar_mul(
                ao_all[:, CPG * j + 64 * h:CPG * j + 64 * h + 64],
                po[:, 65 * hi:65 * hi + 64], rec[:, hi:hi + 1])

    def emit_proj(j):
        t12 = psum_sm(128, 256, BF16)
        nc.tensor.transpose(t12[:, 0:128], ao_all[:, CPG * j:CPG * j + 128],
                            ident[:, :])
        nc.tensor.transpose(t12[0:64, 128:256],
                            ao_all[:, CPG * j + 128:CPG * (j + 1)],
                            ident[:, :])
        a12 = atp.tile([128, 256], BF16, tag="a12", name=f"a12_{j}")
        if j >= 8:
            nc.scalar.copy(a12[:, 0:128], t12[:, 0:128])
            nc.scalar.copy(a12[0:64, 128:256], t12[0:64, 128:256])
        else:
            nc.vector.tensor_copy(a12[:, 0:128], t12[:, 0:128])
            nc.vector.tensor_copy(a12[0:64, 128:256], t12[0:64, 128:256])
        a_hi = a12[:, 0:128]
        a_lo = a12[0:64, 128:256]

        pa = psum_big(128, 512)
        nc.tensor.matmul(pa[:, :], a_hi, wo_hi[:, 0:512],
                         start=True, stop=False)
        nc.tensor.matmul(pa[:, :], a_lo, wo_lo[:, 0:512],
                         start=False, stop=True)
        pb = psum_big(128, 256)
        nc.tensor.matmul(pb[:, :], a_hi, wo_hi[:, 512:768],
                         start=True, stop=False)
        nc.tensor.matmul(pb[:, :], a_lo, wo_lo[:, 512:768],
                         start=False, stop=True)
        ot = otp.tile([128, C], F32, tag="ot", name=f"ot{j}")
        if j >= 8:
            nc.scalar.copy(ot[:, 0:512], pa[:, :])
            nc.scalar.copy(ot[:, 512:768], pb[:, :])
        else:
            nc.vector.tensor_copy(ot[:, 0:512], pa[:, :])
            nc.vector.tensor_copy(ot[:, 512:768], pb[:, :])
        qdma[j % 2].dma_start(out_d[j * 128:(j + 1) * 128, :], ot[:, :])

    # ---- emission order = scheduler priority -------------------------
    # minimal deps for the first score tiles, then scores as early as
    # possible so the ScalarE exp stream starts while qkT/V still run.
    emit_qk_chunk(0, 0)
    emit_qk_chunk(0, 1)
    emit_qk_chunk(1, 0)
    emit_scores(0, 0, chunks=[0])
    emit_scores(2, 0, chunks=[0])
    emit_qk_chunk(0, 2)
    emit_qk_chunk(0, 3)
    emit_scores(0, 0, chunks=[1])
    emit_scores(2, 0, chunks=[1])
    emit_scores(0, 1)
    emit_scores(2, 1)
    for n4 in range(1, 4):
        emit_qk_chunk(1, n4)
    emit_scores(0, 2)
    emit_scores(2, 2)
    for n4 in range(4):
        emit_qk_chunk(2, n4)
    # qk_s[3] = [k1|q1]: partition-swapped copy (SBUF->SBUF DMA)
    nc.gpsimd.dma_start(qk_s[3][0:64, :], qk_s[2][64:128, :])
    nc.gpsimd.dma_start(qk_s[3][64:128, :], qk_s[2][0:64, :])
    emit_v(0)
    # software pipeline: h0/h2 scores run 3 i-groups ahead so the exp
    # stream never starves while h1 (gated on m2+swap) and AV/proj lag;
    # direct-V runs one group ahead of the AV chains that consume it.
    for i in range(NT):
        if i + 3 < NT:
            qlen = T - 128 * (i + 3)
            for ci in range((qlen + 1023) // 1024):
                emit_scores(0, i + 3, chunks=[ci])
                emit_scores(2, i + 3, chunks=[ci])
        emit_scores(1, i)
        if i + 1 < NT:
            emit_v(i + 1)
        emit_av(i, (0, 2))
        emit_av(i, (1,))
        emit_proj(i)


def get_nc():
    if "nc" not in _COMPILED:
        _COMPILED["nc"] = _build_nc()
    return _COMPILED["nc"]


def make_in_maps(x, Wqkv, Wout):
    """Host-side sharding: one input map per core (core = b*G + g)."""
    x = np.asarray(x, dtype=np.float32)
    Wqkv = np.asarray(Wqkv, dtype=np.float32)
    Wout = np.asarray(Wout, dtype=np.float32)

    ident = np.eye(128, dtype=ml_dtypes.bfloat16)
    # mask[k, q] = 1 where k <= q  (valid causal entries of the diag block)
    mask = np.triu(np.ones((128, 128), dtype=np.float32)).astype(
        ml_dtypes.bfloat16)

    in_maps = []
    for b in range(B):
        xt = np.ascontiguousarray(x[b].T).astype(ml_dtypes.bfloat16)
        for g in range(G):
            h0, h1, h2 = (g * HPG + hh for hh in range(HPG))

            def col(kind, hd):
                base = {"q": 0, "k": C, "v": 2 * C}[kind]
                return Wqkv[:, base + hd * DH: base + (hd + 1) * DH]

            # m-tiles: [q0 q2 | k0 k2 | q1 k1]
            wqk = np.concatenate([
                col("q", h0), col("q", h2),
                col("k", h0), col("k", h2),
                col("q", h1), col("k", h1),
            ], axis=1).astype(ml_dtypes.bfloat16)
            wv = np.concatenate(
                [col("v", hd) for hd in (h0, h1, h2)], axis=1,
            ).astype(ml_dtypes.bfloat16)
            wo = np.concatenate(
                [Wout[hd * DH:(hd + 1) * DH, :] for hd in (h0, h1, h2)],
                axis=0,
            ).astype(ml_dtypes.bfloat16)
            in_maps.append({
                "xt": xt, "wqk": np.ascontiguousarray(wqk),
                "wv": np.ascontiguousarray(wv),
                "wo": np.ascontiguousarray(wo),
                "ident": ident, "mask": mask,
            })
    return in_maps


def kernel(x, Wqkv, Wout):
    nc = get_nc()
    in_maps = make_in_maps(x, Wqkv, Wout)
    res = run_bass_kernel_spmd(nc, in_maps, list(range(NCORES))).results
    out = np.zeros((B, T, C), dtype=np.float32)
    for b in range(B):
        for g in range(G):
            out[b] += res[b * G + g]["out"]
    return out


if __name__ == "__main__":
    nc = get_nc()
    print("built + compiled ok")

